# revision 10
# baseline (speedup 1.0000x reference)
"""Trainium2 Bass kernel for nn_BaseModel_20925080666480 (ragged_sequence).

Pipeline (per core, data-parallel over batch: 8 rows/core on 8 cores):
  1. mean over 3 bert layers + CLS drop (streamed from HBM, DVE adds)
  2. ragged subword->word segment-mean via one-hot matmul on TensorE
     (one-hot built on-device from bert2toks with iota + is_equal;
      count normalization folded into the one-hot columns)
  3. input projection x @ W_ih.T + b for both directions (TensorE)
  4. coupled fwd/bwd LSTM scan, gates-on-partitions [128, 16] PSUM tiles
     (gate blocks at 32-aligned partitions: i@0 f@32 o@64 g@96;
      cols 0:8 fwd batch, 8:16 bwd batch), g_in preloaded into PSUM
     via identity matmuls, recurrent matmuls accumulate on top
  5. transpose hs to output layout via TensorE transposes, DMA out

Everything is fp32. Host side only shards/reorders inputs (cheap).
"""

import numpy as np

# problem dims (hardcoded per spec)
B, T, W, D = 64, 512, 256, 768
H = 20
CAP_DIM = 10
IN_DIM = D + CAP_DIM          # 778
PG = 128                      # padded gate rows: i@0, f@32, o@64, g@96
N_CORES = 8
RPC = B // N_CORES            # 8 rows per core
KC = 7                        # phase-2 contraction chunks (6*128 + 11)
LA = 3                        # scan preload lookahead (PSUM bufs = LA+1)

_CACHE = {}


def _build_program(hid_external=True, reps=1):
    """Build the Bass program. hid_external=False makes hiddens an internal
    DRAM tensor (garbage data) for transfer-free timing runs; reps>1 wraps
    the whole body in a hardware loop for wall-clock timing."""
    import concourse.bacc as bacc
    import concourse.mybir as mybir
    import concourse.tile as tile

    f32 = mybir.dt.float32
    i32 = mybir.dt.int32
    ALU = mybir.AluOpType
    ACTF = mybir.ActivationFunctionType

    nc = bacc.Bacc("TRN2", target_bir_lowering=False, debug=False,
                   num_devices=N_CORES)

    if hid_external:
        hid = nc.dram_tensor("hid", [3, RPC, T + 1, D], f32, kind="ExternalInput")
    else:
        hid = nc.dram_tensor("hid", [3, RPC, T + 1, D], f32)
    idsT = nc.dram_tensor("idsT", [128, RPC * 4], f32, kind="ExternalInput")
    capT = nc.dram_tensor("capT", [CAP_DIM + 1, RPC * W], f32, kind="ExternalInput")
    wih = nc.dram_tensor("wih", [128, 2 * KC * PG], f32, kind="ExternalInput")
    whh = nc.dram_tensor("whh", [H, 2 * PG], f32, kind="ExternalInput")
    ident = nc.dram_tensor("ident", [PG, PG], f32, kind="ExternalInput")
    out = nc.dram_tensor("out", [RPC, W, 2 * H], f32, kind="ExternalOutput")

    with tile.TileContext(nc) as tc:
        with (
            tc.tile_pool(name="const", bufs=1) as constp,
            tc.tile_pool(name="stream", bufs=3) as streamp,
            tc.tile_pool(name="onehot", bufs=8) as ohp,
            tc.tile_pool(name="xt", bufs=2) as xtp,
            tc.tile_pool(name="scan", bufs=4) as scanp,
        ):
            # ---- constants (loaded once, outside the timing loop) ----
            iota_i = constp.tile([128, 256], i32)
            nc.gpsimd.iota(iota_i[:], pattern=[[1, 256]], channel_multiplier=0)
            iota_f = constp.tile([128, 256], f32)
            nc.vector.tensor_copy(iota_f[:], iota_i[:])
            threes = constp.tile([128, 1], f32)
            nc.vector.memset(threes[:], 3.0)
            ones_row = constp.tile([1, 128], f32)
            nc.vector.memset(ones_row[:], 1.0)
            id128 = constp.tile([PG, PG], f32)
            nc.sync.dma_start(id128[:], ident[:])
            wih_sb = constp.tile([128, 2 * KC * PG], f32)
            nc.sync.dma_start(wih_sb[:], wih[:])
            whh_sb = constp.tile([H, 2 * PG], f32)
            nc.sync.dma_start(whh_sb[:], whh[:])
            capT_sb = constp.tile([CAP_DIM + 1, RPC * W], f32)
            nc.sync.dma_start(capT_sb[:], capT[:])
            idsT_sb = constp.tile([128, RPC * 4], f32)
            nc.sync.dma_start(idsT_sb[:], idsT[:])

            gin = constp.tile([PG, 2 * W * RPC], f32)  # col = dir*2048 + t*8 + r
            hs_f = constp.tile([H, W * RPC], f32)      # col = r*256 + t
            hs_b = constp.tile([H, W * RPC], f32)

            def phases(psA, psB, psC):
                # ================= phases 1-2, per batch row =================
                for r in range(RPC):
                    # --- normalized one-hot O_norm[t, w] = 1{ids[t]==w}/(3 cnt_w)
                    ots = []
                    for c in range(4):
                        ot = ohp.tile([128, 256], f32, tag="ot")
                        nc.vector.tensor_scalar(
                            ot[:], iota_f[:], idsT_sb[:, r * 4 + c: r * 4 + c + 1],
                            None, op0=ALU.is_equal)
                        ots.append(ot)
                    cnt = psC.tile([1, 256], f32, tag="cnt")
                    for c in range(4):
                        nc.tensor.matmul(cnt[:], threes[:], ots[c][:],
                                         start=(c == 0), stop=(c == 3))
                    recip = ohp.tile([1, 256], f32, tag="recip")
                    nc.vector.reciprocal(recip[:], cnt[:])
                    rb = psC.tile([128, 256], f32, tag="rb")
                    nc.tensor.matmul(rb[:], ones_row[:], recip[:], start=True, stop=True)
                    ons = []
                    for c in range(4):
                        on = ohp.tile([128, 256], f32, tag="on")
                        nc.vector.tensor_tensor(on[:], ots[c][:], rb[:], op=ALU.mult)
                        ons.append(on)

                    # --- stream hiddens, 3-layer sum, segment-sum matmuls
                    xt = xtp.tile([128, KC * 256], f32, tag="xt")
                    pxs = [psA.tile([128, 512], f32, tag="px", name=f"px{j}")
                           for j in range(3)]
                    for c in range(4):
                        ls = []
                        for l in range(3):
                            lt = streamp.tile([128, D], f32, tag=f"l{l}")
                            nc.sync.dma_start(
                                lt[:], hid[l, r, 1 + c * 128: 1 + (c + 1) * 128, :])
                            ls.append(lt)
                        s01 = streamp.tile([128, D], f32, tag="s01")
                        nc.vector.tensor_tensor(s01[:], ls[0][:], ls[1][:], op=ALU.add)
                        sub = streamp.tile([128, D], f32, tag="sub")
                        nc.vector.tensor_tensor(sub[:], s01[:], ls[2][:], op=ALU.add)
                        for dc in range(6):
                            j, half = dc // 2, dc % 2
                            nc.tensor.matmul(
                                pxs[j][:, half * 256:(half + 1) * 256],
                                sub[:, dc * 128:(dc + 1) * 128], ons[c][:],
                                start=(c == 0 and half == 0), stop=(c == 3),
                                skip_group_check=True)
                    # psum -> xT sbuf (word_h.T), plus cap rows + ones row
                    for j in range(3):
                        nc.scalar.copy(xt[:, j * 512:(j + 1) * 512], pxs[j][:])
                    nc.scalar.copy(xt[0:CAP_DIM + 1, 6 * 256:7 * 256],
                                   capT_sb[:, r * W:(r + 1) * W])

                    # --- phase 2: g_in = W_ih @ x.T + b for both dirs
                    gview = gin.rearrange("p (dd t r) -> p dd t r", dd=2, r=RPC)
                    for d in range(2):
                        pg = psB.tile([PG, 256], f32, tag="pg")
                        for kc in range(KC):
                            kk = 128 if kc < 6 else CAP_DIM + 1
                            nc.tensor.matmul(
                                pg[:], wih_sb[0:kk, (d * KC + kc) * PG:(d * KC + kc + 1) * PG],
                                xt[0:kk, kc * 256:(kc + 1) * 256],
                                start=(kc == 0), stop=(kc == KC - 1))
                        # scatter into gin at stride 8 (col = t*8 + r)
                        nc.scalar.copy(gview[:, d, :, r], pg[:])

            def scan(psG, psT):
                # ================= coupled bidirectional LSTM scan =================
                pgs = {}
                hprev = {}
                cprev = None
                ginv = gin.rearrange("p (dd c) -> p dd c", dd=2)
                hfv = hs_f.rearrange("p (r t) -> p r t", r=RPC)
                hbv = hs_b.rearrange("p (r t) -> p r t", r=RPC)
                for it in range(W + LA):
                    # preload g_in for step it into a psum tile (identity matmuls)
                    if it < W:
                        pgt = psG.tile([PG, 16], f32, tag="pgate")
                        tf, tb = it, W - 1 - it
                        nc.tensor.matmul(pgt[:, 0:8], id128[:],
                                         ginv[:, 0, tf * 8:(tf + 1) * 8],
                                         start=True, stop=False, skip_group_check=True)
                        nc.tensor.matmul(pgt[:, 8:16], id128[:],
                                         ginv[:, 1, tb * 8:(tb + 1) * 8],
                                         start=False, stop=False, skip_group_check=True)
                        pgs[it] = pgt
                    s = it - LA
                    if s < 0:
                        continue
                    pgt = pgs.pop(s)
                    tf, tb = s, W - 1 - s
                    if s > 0:
                        nc.tensor.matmul(pgt[:, 0:8], whh_sb[:, 0:PG], hprev["f"][:],
                                         start=False, stop=False, skip_group_check=True)
                        nc.tensor.matmul(pgt[:, 8:16], whh_sb[:, PG:2 * PG], hprev["b"][:],
                                         start=False, stop=True, skip_group_check=True)
                    si = scanp.tile([H, 16], f32, tag="si")
                    nc.scalar.activation(si[:], pgt[0:H, :], ACTF.Sigmoid)
                    sf = scanp.tile([H, 16], f32, tag="sf")
                    nc.scalar.activation(sf[:], pgt[32:32 + H, :], ACTF.Sigmoid)
                    so = scanp.tile([H, 16], f32, tag="so")
                    nc.scalar.activation(so[:], pgt[64:64 + H, :], ACTF.Sigmoid)
                    gt = scanp.tile([H, 16], f32, tag="gt")
                    nc.scalar.activation(gt[:], pgt[96:96 + H, :], ACTF.Tanh)
                    cn = scanp.tile([H, 16], f32, tag="cn")
                    if s > 0:
                        t1 = scanp.tile([H, 16], f32, tag="t1")
                        nc.vector.tensor_tensor(t1[:], si[:], gt[:], op=ALU.mult)
                        t2 = scanp.tile([H, 16], f32, tag="t2")
                        nc.vector.tensor_tensor(t2[:], sf[:], cprev[:], op=ALU.mult)
                        nc.vector.tensor_tensor(cn[:], t1[:], t2[:], op=ALU.add)
                    else:
                        nc.vector.tensor_tensor(cn[:], si[:], gt[:], op=ALU.mult)
                    tct = scanp.tile([H, 16], f32, tag="tct")
                    nc.scalar.activation(tct[:], cn[:], ACTF.Tanh)
                    hf = scanp.tile([H, 8], f32, tag="hf")
                    nc.vector.tensor_tensor(hf[:], so[:, 0:8], tct[:, 0:8], op=ALU.mult)
                    hb = scanp.tile([H, 8], f32, tag="hb")
                    nc.vector.tensor_tensor(hb[:], so[:, 8:16], tct[:, 8:16], op=ALU.mult)
                    # stash h into the hs output buffers (off critical path)
                    nc.gpsimd.tensor_copy(hfv[:, :, tf], hf[:])
                    nc.gpsimd.tensor_copy(hbv[:, :, tb], hb[:])
                    cprev = cn
                    hprev = {"f": hf, "b": hb}

                # ================= output: transpose + DMA =================
                for r in range(RPC):
                    for tb_i in range(2):
                        cols = slice(r * W + tb_i * 128, r * W + (tb_i + 1) * 128)
                        pt = psT.tile([128, 2 * H], f32, tag="pt")
                        nc.tensor.matmul(pt[:, 0:H], hs_f[:, cols], id128[0:H, 0:H],
                                         is_transpose=True, start=True, stop=False,
                                         skip_group_check=True)
                        nc.tensor.matmul(pt[:, H:2 * H], hs_b[:, cols], id128[0:H, 0:H],
                                         is_transpose=True, start=False, stop=True,
                                         skip_group_check=True)
                        hsT = scanp.tile([128, 2 * H], f32, tag="hsT")
                        nc.scalar.copy(hsT[:], pt[:])
                        nc.sync.dma_start(
                            out[r, tb_i * 128:(tb_i + 1) * 128, :], hsT[:])

            def body():
                with (
                    tc.tile_pool(name="psA", bufs=3, space="PSUM") as psA,
                    tc.tile_pool(name="psB", bufs=2, space="PSUM") as psB,
                    tc.tile_pool(name="psC", bufs=1, space="PSUM") as psC,
                ):
                    phases(psA, psB, psC)
                with (
                    tc.tile_pool(name="psG", bufs=LA + 1, space="PSUM") as psG,
                    tc.tile_pool(name="psT", bufs=2, space="PSUM") as psT,
                ):
                    scan(psG, psT)

            if reps > 1:
                with tc.For_i(0, reps):
                    body()
            else:
                body()

    nc.finalize()
    return nc


def _prep_weights(cap_table, w_ih_f, w_hh_f, b_f, w_ih_b, w_hh_b, b_b):
    """Host-side reorder of weights into DMA-friendly layouts (fp32).

    Gate rows are remapped from pytorch order [i,f,g,o] (4x20) to the padded
    device layout [i@0:20, f@32:52, o@64:84, g@96:116] (128 rows).
    """
    dst = np.concatenate([np.arange(0, 20), np.arange(32, 52),
                          np.arange(96, 116), np.arange(64, 84)])  # i,f,g,o lands

    def prep_dir(w_ih, w_hh, b):
        w_ih = np.asarray(w_ih, np.float32)            # [80, 778]
        w_hh = np.asarray(w_hh, np.float32)            # [80, 20]
        b = np.asarray(b, np.float32)                  # [80]
        w_ih_p = np.zeros((PG, IN_DIM), np.float32)
        w_hh_p = np.zeros((PG, H), np.float32)
        b_p = np.zeros(PG, np.float32)
        w_ih_p[dst] = w_ih
        w_hh_p[dst] = w_hh
        b_p[dst] = b
        wihT = np.concatenate([w_ih_p.T, b_p[None, :]], 0)  # [779, 128]
        wihT = np.pad(wihT, ((0, KC * 128 - wihT.shape[0]), (0, 0)))
        chunks = wihT.reshape(KC, 128, PG).transpose(1, 0, 2).reshape(128, KC * PG)
        return chunks, w_hh_p.T.copy()                  # [128, 896], [20, 128]

    cf, hf = prep_dir(w_ih_f, w_hh_f, b_f)
    cb, hb = prep_dir(w_ih_b, w_hh_b, b_b)
    wih_host = np.ascontiguousarray(np.concatenate([cf, cb], axis=1))   # [128, 1792]
    whh_host = np.ascontiguousarray(np.concatenate([hf, hb], axis=1))   # [20, 256]
    ident = np.eye(PG, dtype=np.float32)
    return wih_host, whh_host, ident


def kernel(**inputs) -> np.ndarray:
    from concourse.bass_utils import run_bass_kernel_spmd

    hiddens = np.ascontiguousarray(np.asarray(inputs["hiddens"], np.float32))
    bert2toks = np.asarray(inputs["bert2toks"]).astype(np.int64)
    cap_inds = np.asarray(inputs["cap_inds"]).astype(np.int64)
    cap_table = np.asarray(inputs["cap_table"], np.float32)

    wih_host, whh_host, ident = _prep_weights(
        cap_table, inputs["w_ih_f"], inputs["w_hh_f"], inputs["b_f"],
        inputs["w_ih_b"], inputs["w_hh_b"], inputs["b_b"])

    if "nc" not in _CACHE:
        _CACHE["nc"] = _build_program(hid_external=True, reps=1)
    nc = _CACHE["nc"]

    cap_emb = cap_table[cap_inds]                      # [64, 256, 10]
    in_maps = []
    for c in range(N_CORES):
        rows = slice(c * RPC, (c + 1) * RPC)
        ids = bert2toks[rows]                          # [8, 512]
        idsT = ids.reshape(RPC, 4, 128).transpose(2, 0, 1).reshape(128, RPC * 4)
        capT = np.concatenate(
            [cap_emb[rows].transpose(2, 0, 1).reshape(CAP_DIM, RPC * W),
             np.ones((1, RPC * W), np.float32)], axis=0)
        in_maps.append({
            "hid": np.ascontiguousarray(hiddens[:, rows]),
            "idsT": np.ascontiguousarray(idsT.astype(np.float32)),
            "capT": np.ascontiguousarray(capT.astype(np.float32)),
            "wih": wih_host,
            "whh": whh_host,
            "ident": ident,
        })

    res = run_bass_kernel_spmd(nc, in_maps, list(range(N_CORES)))
    return np.concatenate([res.results[c]["out"] for c in range(N_CORES)],
                          axis=0).astype(np.float32)


# revision 22
# speedup vs baseline: 1.1471x; 1.1471x over previous
"""Trainium2 Bass kernel for nn_BaseModel_20925080666480 (ragged_sequence).

Pipeline (per core, data-parallel over batch: 8 rows/core on 8 cores):
  1. mean over 3 bert layers + CLS drop (streamed from HBM, DVE adds)
  2. ragged subword->word segment-mean via one-hot matmul on TensorE
     (one-hot built on-device from bert2toks with iota + is_equal;
      count normalization folded into the one-hot columns)
  3. input projection x @ W_ih.T + b for both directions (TensorE)
  4. coupled fwd/bwd LSTM scan, gates-on-partitions [128, 16] PSUM tiles
     (gate blocks at 32-aligned partitions: i@0 f@32 o@64 g@96;
      cols 0:8 fwd batch, 8:16 bwd batch), g_in preloaded into PSUM
     via identity matmuls, recurrent matmuls accumulate on top
  5. transpose hs to output layout via TensorE transposes, DMA out

Everything is fp32. Host side only shards/reorders inputs (cheap).
"""

import numpy as np

# problem dims (hardcoded per spec)
B, T, W, D = 64, 512, 256, 768
H = 20
CAP_DIM = 10
IN_DIM = D + CAP_DIM          # 778
PG = 128                      # padded gate rows: f@0, o@32, i@64, g@96
N_CORES = 8
RPC = B // N_CORES            # 8 rows per core
KC = 7                        # phase-2 contraction chunks (6*128 + 11)
LA = 2                        # scan preload lookahead (pgate shares px slots)

_CACHE = {}


def _build_program(hid_external=True, reps=1, parts="all", abl=(), nbody=1):
    """Build the Bass program. hid_external=False makes hiddens an internal
    DRAM tensor (garbage data) for transfer-free timing runs; reps>1 wraps
    the whole body in a hardware loop for wall-clock timing."""
    import concourse.bacc as bacc
    import concourse.mybir as mybir
    import concourse.tile as tile
    import bass_rust

    def apmod(ap, offset, dims):
        c = ap.copy()
        c.offset = offset
        c.ap = bass_rust.VecI64Pair(dims)
        return c

    f32 = mybir.dt.float32
    i32 = mybir.dt.int32
    ALU = mybir.AluOpType
    ACTF = mybir.ActivationFunctionType

    nc = bacc.Bacc("TRN2", target_bir_lowering=False, debug=False,
                   num_devices=N_CORES)

    if hid_external:
        hid = nc.dram_tensor("hid", [3, RPC, T + 1, D], f32, kind="ExternalInput")
    else:
        hid = nc.dram_tensor("hid", [3, RPC, T + 1, D], f32)
    idsT = nc.dram_tensor("idsT", [128, RPC * 4], f32, kind="ExternalInput")
    capT = nc.dram_tensor("capT", [CAP_DIM + 1, RPC * W], f32, kind="ExternalInput")
    wih = nc.dram_tensor("wih", [128, 2 * KC * PG], f32, kind="ExternalInput")
    whh = nc.dram_tensor("whh", [H, 2 * PG], f32, kind="ExternalInput")
    ident = nc.dram_tensor("ident", [PG, PG], f32, kind="ExternalInput")
    out = nc.dram_tensor("out", [RPC, W, 2 * H], f32, kind="ExternalOutput")

    with tile.TileContext(nc) as tc:
        with (
            tc.tile_pool(name="const", bufs=1) as constp,
            tc.tile_pool(name="stream", bufs=3) as streamp,
            tc.tile_pool(name="onehot", bufs=8) as ohp,
            tc.tile_pool(name="xt", bufs=8) as xtp,
            tc.tile_pool(name="scan", bufs=6) as scanp,
            tc.tile_pool(name="psP", bufs=1, space="PSUM") as psP,
        ):
            # ---- constants (loaded once, outside the timing loop) ----
            iota_i = constp.tile([128, 256], i32)
            nc.gpsimd.iota(iota_i[:], pattern=[[1, 256]], channel_multiplier=0)
            iota_f = constp.tile([128, 256], f32)
            nc.vector.tensor_copy(iota_f[:], iota_i[:])
            threes = constp.tile([128, 1], f32)
            nc.vector.memset(threes[:], 3.0)
            ones_row = constp.tile([1, 128], f32)
            nc.vector.memset(ones_row[:], 1.0)
            idm = constp.tile([PG, PG], f32)
            nc.sync.dma_start(idm[:], ident[:])
            wih_sb = constp.tile([128, 2 * KC * PG], f32)
            nc.sync.dma_start(wih_sb[:], wih[:])
            whh_sb = constp.tile([H, 2 * PG], f32)
            nc.sync.dma_start(whh_sb[:], whh[:])
            capT_sb = constp.tile([CAP_DIM + 1, RPC * W], f32)
            nc.sync.dma_start(capT_sb[:], capT[:])
            idsT_sb = constp.tile([128, RPC * 4], f32)
            nc.sync.dma_start(idsT_sb[:], idsT[:])

            gin = constp.tile([PG, 2 * W * RPC], f32)  # fwd col t*8+r; bwd col W*RPC + u*8+r (u = scan step)
            hs = constp.tile([H, 2 * W * RPC], f32)    # fwd col r*W+t; bwd col W*RPC + r*W+t
            hsfv = hs.rearrange("p (dd r t) -> p dd r t", dd=2, r=RPC)[:, 0]
            hsbv = hs.rearrange("p (dd r t) -> p dd r t", dd=2, r=RPC)[:, 1]

            def phases(psP):
                # ================= phases 1-2 =================
                # PE warm-up: ~3.5us of back-to-back matmuls releases the HAM
                # clock gate (1.2 -> 2.4 GHz) before the phase-1 matmul burst
                wps = psP.tile([128, 256], f32, tag="cr", bufs=1, name="warm")
                for _ in range(16):
                    nc.tensor.matmul(wps[:, 0:128], idm[:, 0:128], idm[:, 0:128],
                                     start=True, stop=True, skip_group_check=True)
                xts = []
                for r in range(RPC):
                    # --- normalized one-hot O_norm[t, w] = 1{ids[t]==w}/(3 cnt_w)
                    ots = []
                    for c in range(4):
                        ot = ohp.tile([128, 256], f32, tag="ot")
                        nc.vector.tensor_scalar(
                            ot[:], iota_f[:], idsT_sb[:, r * 4 + c: r * 4 + c + 1],
                            None, op0=ALU.is_equal)
                        ots.append(ot)
                    cnt = psP.tile([1, 256], f32, tag="cr", bufs=1, name="cnt")
                    for c in range(4):
                        nc.tensor.matmul(cnt[:], threes[:], ots[c][:],
                                         start=(c == 0), stop=(c == 3))
                    recip = ohp.tile([1, 256], f32, tag="recip")
                    nc.vector.reciprocal(recip[:], cnt[:])
                    rb = psP.tile([128, 256], f32, tag="cr", bufs=1, name="rb")
                    nc.tensor.matmul(rb[:], ones_row[:], recip[:], start=True, stop=True)
                    ons = []
                    for c in range(4):
                        on = ohp.tile([128, 256], f32, tag="on")
                        nc.vector.tensor_tensor(on[:], ots[c][:], rb[:], op=ALU.mult)
                        ons.append(on)

                    # --- stream hiddens, 3-layer sum, segment-sum matmuls
                    xt = xtp.tile([128, KC * 256], f32, tag="xt")
                    pxs = [psP.tile([128, 512], f32, tag="px", bufs=3, name=f"px{j}")
                           for j in range(3)]
                    for c in range(4):
                        lt3 = streamp.tile([128, 3 * D], f32, tag="lt3")
                        nc.sync.dma_start(
                            lt3[:],
                            hid[:, r, 1 + c * 128: 1 + (c + 1) * 128, :].rearrange(
                                "l p e -> p l e"))
                        s01 = streamp.tile([128, D], f32, tag="s01")
                        nc.vector.tensor_tensor(s01[:], lt3[:, 0:D], lt3[:, D:2 * D],
                                                op=ALU.add)
                        sub = streamp.tile([128, D], f32, tag="sub")
                        nc.vector.tensor_tensor(sub[:], s01[:], lt3[:, 2 * D:3 * D],
                                                op=ALU.add)
                        for dc in range(6):
                            j, half = dc // 2, dc % 2
                            nc.tensor.matmul(
                                pxs[j][:, half * 256:(half + 1) * 256],
                                sub[:, dc * 128:(dc + 1) * 128], ons[c][:],
                                start=(c == 0 and half == 0), stop=(c == 3),
                                skip_group_check=True)
                    # psum -> xT sbuf (word_h.T), plus cap rows + ones row
                    for j in range(3):
                        nc.scalar.copy(xt[:, j * 512:(j + 1) * 512], pxs[j][:])
                    nc.scalar.copy(xt[0:CAP_DIM + 1, 6 * 256:7 * 256],
                                   capT_sb[:, r * W:(r + 1) * W])

                    xts.append(xt)
                # --- phase 2 (grouped): g_in = W_ih @ x.T + b, ldw amortized
                # over rows; bwd (d=1) written time-reversed for burst preloads
                gview = gin.rearrange("p (dd t r) -> p dd t r", dd=2, r=RPC)
                for grp in range(2):
                    rows = range(grp * 4, grp * 4 + 4)
                    pgs2 = {}
                    for d in range(2):
                        for r in rows:
                            pgs2[(d, r)] = psP.tile([PG, 256], f32, tag="pg", bufs=4,
                                                    name=f"pg{d}{r}")
                    for d in range(2):
                        for kc in range(KC):
                            kk = 128 if kc < 6 else CAP_DIM + 1
                            for r in rows:
                                nc.tensor.matmul(
                                    pgs2[(d, r)][:],
                                    wih_sb[0:kk, (d * KC + kc) * PG:(d * KC + kc + 1) * PG],
                                    xts[r][0:kk, kc * 256:(kc + 1) * 256],
                                    start=(kc == 0), stop=(kc == KC - 1),
                                    skip_group_check=True)
                    for d in range(2):
                        for r in rows:
                            if d == 0:
                                nc.scalar.copy(gview[:, 0, :, r], pgs2[(d, r)][:])
                            else:
                                dst = apmod(gin[:, 0:256],
                                            W * RPC + (W - 1) * RPC + r,
                                            [[2 * W * RPC, PG], [-RPC, W]])
                                nc.scalar.copy(dst, pgs2[(d, r)][:])

            def scan(psP, standalone=False):
                # ================= coupled bidirectional LSTM scan =================
                # g_in bursts: BL steps per PSUM bank tile [PG, BL*16]
                # (cols j*16+0:8 fwd step s0+j, j*16+8:16 bwd step s0+j).
                # hn writes go straight into hs (fwd region col r*W+t,
                # bwd region W*RPC + r*W + t).
                if standalone:
                    nc.gpsimd.memset(gin[:], 0.01)
                BL = 8
                NB = W // BL
                bursts = {}

                def emit_burst(b):
                    pgm = psP.tile([PG, BL * 16], f32, tag="px", bufs=3,
                                   name=f"pgm{b}")
                    s0 = b * BL
                    # fwd: out cols {16j..16j+8}, rhs contiguous gin cols
                    outf = apmod(pgm[:, 0:16], 0, [[BL * 16, PG], [16, BL], [1, 8]])
                    nc.tensor.matmul(outf, idm[:],
                                     gin[:, s0 * 8:(s0 + BL) * 8],
                                     start=True, stop=False, skip_group_check=True)
                    # bwd (gin bwd region stored by scan-step index already)
                    outb = apmod(pgm[:, 0:16], 8, [[BL * 16, PG], [16, BL], [1, 8]])
                    nc.tensor.matmul(outb, idm[:],
                                     gin[:, W * RPC + s0 * 8: W * RPC + (s0 + BL) * 8],
                                     start=False, stop=False, skip_group_check=True)
                    bursts[b] = pgm

                emit_burst(0)
                emit_burst(1)
                hprev = {}
                cprev = None
                for s in range(W):
                    j = s % BL
                    if j == 0 and s // BL + 2 < NB:
                        emit_burst(s // BL + 2)
                    pgm = bursts[s // BL]
                    pgt = pgm[:, j * 16:(j + 1) * 16]
                    tf, tb = s, W - 1 - s
                    if s > 0 and "norecur" not in abl:
                        nc.tensor.matmul(pgt[:, 0:8], whh_sb[:, 0:PG], hprev["f"],
                                         start=False, stop=False, skip_group_check=True)
                        nc.tensor.matmul(pgt[:, 8:16], whh_sb[:, PG:2 * PG], hprev["b"],
                                         start=False, stop=(j == BL - 1),
                                         skip_group_check=True)
                    # sig rows (junk pads): f@0:20, o@32:52, i@64:84, 2g@96:116
                    sig = scanp.tile([116, 16], f32, tag="sig")
                    nc.scalar.activation(sig[:], pgt[0:116, :], ACTF.Sigmoid)
                    gt96 = scanp.tile([96, 16], f32, tag="gt96")
                    # tanh(g) = 2*sigmoid(2g) - 1 (g pre-scaled by 2 in weights)
                    nc.vector.tensor_scalar(gt96[64:84, :], sig[96:116, :], 2.0, -1.0,
                                            op0=ALU.mult, op1=ALU.add)
                    cn = scanp.tile([H, 16], f32, tag="cn")
                    if s > 0:
                        t2 = scanp.tile([H, 16], f32, tag="t2")
                        nc.vector.tensor_tensor(t2[:], sig[0:20, :], cprev[:], op=ALU.mult)
                        t1 = scanp.tile([H, 16], f32, tag="t1")
                        nc.vector.tensor_tensor(t1[:], sig[64:84, :], gt96[64:84, :], op=ALU.mult)
                        nc.vector.tensor_tensor(cn[:], t1[:], t2[:], op=ALU.add)
                    else:
                        nc.vector.tensor_tensor(cn[:], sig[64:84, :], gt96[64:84, :], op=ALU.mult)
                    tc64 = scanp.tile([64, 16], f32, tag="tc64")
                    nc.scalar.activation(tc64[32:52, :], cn[:], ACTF.Tanh)
                    # h written straight into hs: fwd col r*W+tf, bwd col
                    # W*RPC + r*W + tb; strides: per-col-block delta
                    hdst = apmod(hs[:, 0:16], tf,
                                 [[2 * W * RPC, H], [W * RPC + tb - tf, 2], [W, 8]])
                    nc.vector.tensor_tensor(hdst, sig[32:52, :], tc64[32:52, :],
                                            op=ALU.mult)
                    cprev = cn
                    hprev = {"f": hsfv[:, :, tf], "b": hsbv[:, :, tb]}

                # ================= output: transpose + DMA =================
                for r in range(RPC):
                    for tb_i in range(2):
                        cols = slice(r * W + tb_i * 128, r * W + (tb_i + 1) * 128)
                        colsb = slice(W * RPC + r * W + tb_i * 128,
                                      W * RPC + r * W + (tb_i + 1) * 128)
                        pt = psP.tile([128, 2 * H], f32, tag="pg", bufs=4, name="pt")
                        nc.tensor.matmul(pt[:, 0:H], hs[:, cols], idm[0:H, 0:H],
                                         is_transpose=True, start=True, stop=False,
                                         skip_group_check=True)
                        nc.tensor.matmul(pt[:, H:2 * H], hs[:, colsb], idm[0:H, 0:H],
                                         is_transpose=True, start=False, stop=True,
                                         skip_group_check=True)
                        hsT = scanp.tile([128, 2 * H], f32, tag="hsT")
                        nc.scalar.copy(hsT[:], pt[:])
                        nc.sync.dma_start(
                            out[r, tb_i * 128:(tb_i + 1) * 128, :], hsT[:])

            def body():
                if parts in ("all", "phases"):
                    phases(psP)
                if parts in ("all", "scan"):
                    scan(psP, standalone=(parts == "scan"))

            if reps > 1:
                with tc.For_i(0, reps):
                    for _ in range(nbody):
                        body()
            else:
                body()

    nc.finalize()
    return nc


def _prep_weights(cap_table, w_ih_f, w_hh_f, b_f, w_ih_b, w_hh_b, b_b):
    """Host-side reorder of weights into DMA-friendly layouts (fp32).

    Gate rows are remapped from pytorch order [i,f,g,o] (4x20) to the padded
    device layout [i@0:20, f@32:52, o@64:84, g@96:116] (128 rows).
    """
    dst = np.concatenate([np.arange(64, 84), np.arange(0, 20),
                          np.arange(96, 116), np.arange(32, 52)])  # i,f,g,o lands

    def prep_dir(w_ih, w_hh, b):
        w_ih = np.asarray(w_ih, np.float32)            # [80, 778]
        w_hh = np.asarray(w_hh, np.float32)            # [80, 20]
        b = np.asarray(b, np.float32)                  # [80]
        w_ih_p = np.zeros((PG, IN_DIM), np.float32)
        w_hh_p = np.zeros((PG, H), np.float32)
        b_p = np.zeros(PG, np.float32)
        w_ih_p[dst] = w_ih
        w_hh_p[dst] = w_hh
        b_p[dst] = b
        w_ih_p[96:116] *= 2.0   # tanh(g) via 2*sigmoid(2g) - 1
        w_hh_p[96:116] *= 2.0
        b_p[96:116] *= 2.0
        wihT = np.concatenate([w_ih_p.T, b_p[None, :]], 0)  # [779, 128]
        wihT = np.pad(wihT, ((0, KC * 128 - wihT.shape[0]), (0, 0)))
        chunks = wihT.reshape(KC, 128, PG).transpose(1, 0, 2).reshape(128, KC * PG)
        return chunks, w_hh_p.T.copy()                  # [128, 896], [20, 128]

    cf, hf = prep_dir(w_ih_f, w_hh_f, b_f)
    cb, hb = prep_dir(w_ih_b, w_hh_b, b_b)
    wih_host = np.ascontiguousarray(np.concatenate([cf, cb], axis=1))   # [128, 1792]
    whh_host = np.ascontiguousarray(np.concatenate([hf, hb], axis=1))   # [20, 256]
    ident = np.eye(PG, dtype=np.float32)
    return wih_host, whh_host, ident


def kernel(**inputs) -> np.ndarray:
    from concourse.bass_utils import run_bass_kernel_spmd

    hiddens = np.ascontiguousarray(np.asarray(inputs["hiddens"], np.float32))
    bert2toks = np.asarray(inputs["bert2toks"]).astype(np.int64)
    cap_inds = np.asarray(inputs["cap_inds"]).astype(np.int64)
    cap_table = np.asarray(inputs["cap_table"], np.float32)

    wih_host, whh_host, ident = _prep_weights(
        cap_table, inputs["w_ih_f"], inputs["w_hh_f"], inputs["b_f"],
        inputs["w_ih_b"], inputs["w_hh_b"], inputs["b_b"])

    if "nc" not in _CACHE:
        _CACHE["nc"] = _build_program(hid_external=True, reps=1)
    nc = _CACHE["nc"]

    cap_emb = cap_table[cap_inds]                      # [64, 256, 10]
    in_maps = []
    for c in range(N_CORES):
        rows = slice(c * RPC, (c + 1) * RPC)
        ids = bert2toks[rows]                          # [8, 512]
        idsT = ids.reshape(RPC, 4, 128).transpose(2, 0, 1).reshape(128, RPC * 4)
        capT = np.concatenate(
            [cap_emb[rows].transpose(2, 0, 1).reshape(CAP_DIM, RPC * W),
             np.ones((1, RPC * W), np.float32)], axis=0)
        in_maps.append({
            "hid": np.ascontiguousarray(hiddens[:, rows]),
            "idsT": np.ascontiguousarray(idsT.astype(np.float32)),
            "capT": np.ascontiguousarray(capT.astype(np.float32)),
            "wih": wih_host,
            "whh": whh_host,
            "ident": ident,
        })

    res = run_bass_kernel_spmd(nc, in_maps, list(range(N_CORES)))
    return np.concatenate([res.results[c]["out"] for c in range(N_CORES)],
                          axis=0).astype(np.float32)


# revision 24
# speedup vs baseline: 1.2225x; 1.0658x over previous
"""Trainium2 Bass kernel for nn_BaseModel_20925080666480 (ragged_sequence).

Pipeline (per core, data-parallel over batch: 8 rows/core on 8 cores):
  1. mean over 3 bert layers + CLS drop (streamed from HBM, DVE adds)
  2. ragged subword->word segment-mean via one-hot matmul on TensorE
     (one-hot built on-device from bert2toks with iota + is_equal;
      count normalization folded into the one-hot columns)
  3. input projection x @ W_ih.T + b for both directions (TensorE)
  4. coupled fwd/bwd LSTM scan, gates-on-partitions [128, 16] PSUM tiles
     (gate blocks at 32-aligned partitions: i@0 f@32 o@64 g@96;
      cols 0:8 fwd batch, 8:16 bwd batch), g_in preloaded into PSUM
     via identity matmuls, recurrent matmuls accumulate on top
  5. transpose hs to output layout via TensorE transposes, DMA out

Everything is fp32. Host side only shards/reorders inputs (cheap).
"""

import numpy as np

# problem dims (hardcoded per spec)
B, T, W, D = 64, 512, 256, 768
H = 20
CAP_DIM = 10
IN_DIM = D + CAP_DIM          # 778
PG = 128                      # padded gate rows: f@0, o@32, i@64, g@96
N_CORES = 8
RPC = B // N_CORES            # 8 rows per core
KC = 7                        # phase-2 contraction chunks (6*128 + 11)
LA = 2                        # scan preload lookahead (pgate shares px slots)

_CACHE = {}


def _build_program(hid_external=True, reps=1, parts="all", abl=(), nbody=1):
    """Build the Bass program. hid_external=False makes hiddens an internal
    DRAM tensor (garbage data) for transfer-free timing runs; reps>1 wraps
    the whole body in a hardware loop for wall-clock timing."""
    import concourse.bacc as bacc
    import concourse.mybir as mybir
    import concourse.tile as tile
    import bass_rust

    def apmod(ap, offset, dims):
        c = ap.copy()
        c.offset = offset
        c.ap = bass_rust.VecI64Pair(dims)
        return c

    f32 = mybir.dt.float32
    i32 = mybir.dt.int32
    ALU = mybir.AluOpType
    ACTF = mybir.ActivationFunctionType

    nc = bacc.Bacc("TRN2", target_bir_lowering=False, debug=False,
                   num_devices=N_CORES)

    if hid_external:
        hid = nc.dram_tensor("hid", [3, RPC, T + 1, D], f32, kind="ExternalInput")
    else:
        hid = nc.dram_tensor("hid", [3, RPC, T + 1, D], f32)
    idsT = nc.dram_tensor("idsT", [128, RPC * 4], f32, kind="ExternalInput")
    capT = nc.dram_tensor("capT", [CAP_DIM + 1, RPC * W], f32, kind="ExternalInput")
    wih = nc.dram_tensor("wih", [128, 2 * KC * PG], f32, kind="ExternalInput")
    whh = nc.dram_tensor("whh", [H, 2 * PG], f32, kind="ExternalInput")
    ident = nc.dram_tensor("ident", [PG, PG], f32, kind="ExternalInput")
    out = nc.dram_tensor("out", [RPC, W, 2 * H], f32, kind="ExternalOutput")

    with tile.TileContext(nc) as tc:
        with (
            tc.tile_pool(name="const", bufs=1) as constp,
            tc.tile_pool(name="stream", bufs=3) as streamp,
            tc.tile_pool(name="onehot", bufs=8) as ohp,
            tc.tile_pool(name="xt", bufs=8) as xtp,
            tc.tile_pool(name="scan", bufs=6) as scanp,
            tc.tile_pool(name="psP", bufs=1, space="PSUM") as psP,
        ):
            # ---- constants (loaded once, outside the timing loop) ----
            iota_i = constp.tile([128, 256], i32)
            nc.gpsimd.iota(iota_i[:], pattern=[[1, 256]], channel_multiplier=0)
            iota_f = constp.tile([128, 256], f32)
            nc.vector.tensor_copy(iota_f[:], iota_i[:])
            threes = constp.tile([128, 1], f32)
            nc.vector.memset(threes[:], 3.0)
            ones_row = constp.tile([1, 128], f32)
            nc.vector.memset(ones_row[:], 1.0)
            idm = constp.tile([PG, PG], f32)
            nc.sync.dma_start(idm[:], ident[:])
            wih_sb = constp.tile([128, 2 * KC * PG], f32)
            nc.sync.dma_start(wih_sb[:], wih[:])
            whh_sb = constp.tile([H, 2 * PG], f32)
            nc.sync.dma_start(whh_sb[:], whh[:])
            capT_sb = constp.tile([CAP_DIM + 1, RPC * W], f32)
            nc.sync.dma_start(capT_sb[:], capT[:])
            idsT_sb = constp.tile([128, RPC * 4], f32)
            nc.sync.dma_start(idsT_sb[:], idsT[:])

            gin = constp.tile([PG, 2 * W * RPC], f32)  # fwd col t*8+r; bwd col W*RPC + u*8+r (u = scan step)
            hs = constp.tile([H, 2 * W * RPC], f32)    # fwd col r*W+t; bwd col W*RPC + r*W+t
            hsfv = hs.rearrange("p (dd r t) -> p dd r t", dd=2, r=RPC)[:, 0]
            hsbv = hs.rearrange("p (dd r t) -> p dd r t", dd=2, r=RPC)[:, 1]

            def phases(psP):
                # ================= phases 1-2 =================
                # PE warm-up: ~3.5us of back-to-back matmuls releases the HAM
                # clock gate (1.2 -> 2.4 GHz) before the phase-1 matmul burst
                wps = psP.tile([128, 256], f32, tag="cr", bufs=1, name="warm")
                for _ in range(16):
                    nc.tensor.matmul(wps[:, 0:128], idm[:, 0:128], idm[:, 0:128],
                                     start=True, stop=True, skip_group_check=True)
                xts = []
                for r in range(RPC):
                    # --- normalized one-hot O_norm[t, w] = 1{ids[t]==w}/(3 cnt_w)
                    ots = []
                    for c in range(4):
                        ot = ohp.tile([128, 256], f32, tag="ot")
                        nc.vector.tensor_scalar(
                            ot[:], iota_f[:], idsT_sb[:, r * 4 + c: r * 4 + c + 1],
                            None, op0=ALU.is_equal)
                        ots.append(ot)
                    cnt = psP.tile([1, 256], f32, tag="cr", bufs=1, name="cnt")
                    for c in range(4):
                        nc.tensor.matmul(cnt[:], threes[:], ots[c][:],
                                         start=(c == 0), stop=(c == 3))
                    recip = ohp.tile([1, 256], f32, tag="recip")
                    nc.vector.reciprocal(recip[:], cnt[:])
                    rb = psP.tile([128, 256], f32, tag="cr", bufs=1, name="rb")
                    nc.tensor.matmul(rb[:], ones_row[:], recip[:], start=True, stop=True)
                    ons = []
                    for c in range(4):
                        on = ohp.tile([128, 256], f32, tag="on")
                        nc.vector.tensor_tensor(on[:], ots[c][:], rb[:], op=ALU.mult)
                        ons.append(on)

                    # --- stream hiddens, 3-layer sum, segment-sum matmuls
                    xt = xtp.tile([128, KC * 256], f32, tag="xt")
                    pxs = [psP.tile([128, 512], f32, tag="px", bufs=3, name=f"px{j}")
                           for j in range(3)]
                    for c in range(4):
                        lt3 = streamp.tile([128, 3 * D], f32, tag="lt3")
                        nc.sync.dma_start(
                            lt3[:],
                            hid[:, r, 1 + c * 128: 1 + (c + 1) * 128, :].rearrange(
                                "l p e -> p l e"))
                        s01 = streamp.tile([128, D], f32, tag="s01")
                        nc.vector.tensor_tensor(s01[:], lt3[:, 0:D], lt3[:, D:2 * D],
                                                op=ALU.add)
                        sub = streamp.tile([128, D], f32, tag="sub")
                        nc.vector.tensor_tensor(sub[:], s01[:], lt3[:, 2 * D:3 * D],
                                                op=ALU.add)
                        for dc in range(6):
                            j, half = dc // 2, dc % 2
                            nc.tensor.matmul(
                                pxs[j][:, half * 256:(half + 1) * 256],
                                sub[:, dc * 128:(dc + 1) * 128], ons[c][:],
                                start=(c == 0 and half == 0), stop=(c == 3),
                                skip_group_check=True)
                    # psum -> xT sbuf (word_h.T), plus cap rows + ones row
                    for j in range(3):
                        nc.scalar.copy(xt[:, j * 512:(j + 1) * 512], pxs[j][:])
                    nc.scalar.copy(xt[0:CAP_DIM + 1, 6 * 256:7 * 256],
                                   capT_sb[:, r * W:(r + 1) * W])

                    xts.append(xt)
                # --- phase 2 (grouped): g_in = W_ih @ x.T + b, ldw amortized
                # over rows; bwd (d=1) written time-reversed for burst preloads
                gview = gin.rearrange("p (dd t r) -> p dd t r", dd=2, r=RPC)
                for grp in range(2):
                    rows = range(grp * 4, grp * 4 + 4)
                    pgs2 = {}
                    for d in range(2):
                        for r in rows:
                            pgs2[(d, r)] = psP.tile([PG, 256], f32, tag="pg", bufs=4,
                                                    name=f"pg{d}{r}")
                    for d in range(2):
                        for kc in range(KC):
                            kk = 128 if kc < 6 else CAP_DIM + 1
                            for r in rows:
                                nc.tensor.matmul(
                                    pgs2[(d, r)][:],
                                    wih_sb[0:kk, (d * KC + kc) * PG:(d * KC + kc + 1) * PG],
                                    xts[r][0:kk, kc * 256:(kc + 1) * 256],
                                    start=(kc == 0), stop=(kc == KC - 1),
                                    skip_group_check=True)
                    for d in range(2):
                        for r in rows:
                            if d == 0:
                                nc.scalar.copy(gview[:, 0, :, r], pgs2[(d, r)][:])
                            else:
                                dst = apmod(gin[:, 0:256],
                                            W * RPC + (W - 1) * RPC + r,
                                            [[2 * W * RPC, PG], [-RPC, W]])
                                nc.scalar.copy(dst, pgs2[(d, r)][:])

            def scan(psP, standalone=False):
                # ================= coupled bidirectional LSTM scan =================
                # g_in bursts: BL steps per PSUM bank tile [PG, BL*16]
                # (cols j*16+0:8 fwd step s0+j, j*16+8:16 bwd step s0+j).
                # hn writes go straight into hs (fwd region col r*W+t,
                # bwd region W*RPC + r*W + t).
                if standalone:
                    nc.gpsimd.memset(gin[:], 0.01)
                BL = 16
                NB = W // BL
                bursts = {}

                def emit_burst(b):
                    pgm = psP.tile([PG, BL * 16], f32, tag="px", bufs=3,
                                   name=f"pgm{b}")
                    s0 = b * BL
                    # fwd: out cols {16j..16j+8}, rhs contiguous gin cols
                    outf = apmod(pgm[:, 0:16], 0, [[BL * 16, PG], [16, BL], [1, 8]])
                    nc.tensor.matmul(outf, idm[:],
                                     gin[:, s0 * 8:(s0 + BL) * 8],
                                     start=True, stop=False, skip_group_check=True)
                    # bwd (gin bwd region stored by scan-step index already)
                    outb = apmod(pgm[:, 0:16], 8, [[BL * 16, PG], [16, BL], [1, 8]])
                    nc.tensor.matmul(outb, idm[:],
                                     gin[:, W * RPC + s0 * 8: W * RPC + (s0 + BL) * 8],
                                     start=False, stop=False, skip_group_check=True)
                    bursts[b] = pgm

                emit_burst(0)
                emit_burst(1)
                hprev = {}
                cprev = None
                for s in range(W):
                    j = s % BL
                    if j == 0 and s // BL + 2 < NB:
                        emit_burst(s // BL + 2)
                    pgm = bursts[s // BL]
                    pgt = pgm[:, j * 16:(j + 1) * 16]
                    tf, tb = s, W - 1 - s
                    if s > 0 and "norecur" not in abl:
                        nc.tensor.matmul(pgt[:, 0:8], whh_sb[:, 0:PG], hprev["f"],
                                         start=False, stop=False, skip_group_check=True)
                        nc.tensor.matmul(pgt[:, 8:16], whh_sb[:, PG:2 * PG], hprev["b"],
                                         start=False, stop=(j == BL - 1),
                                         skip_group_check=True)
                    # sig rows (junk pads): f@0:20, o@32:52, i@64:84, 2g@96:116
                    sig = scanp.tile([116, 16], f32, tag="sig")
                    nc.scalar.activation(sig[:], pgt[0:116, :], ACTF.Sigmoid)
                    gt96 = scanp.tile([96, 16], f32, tag="gt96")
                    # tanh(g) = 2*sigmoid(2g) - 1 (g pre-scaled by 2 in weights)
                    nc.vector.tensor_scalar(gt96[64:84, :], sig[96:116, :], 2.0, -1.0,
                                            op0=ALU.mult, op1=ALU.add)
                    cn = scanp.tile([H, 16], f32, tag="cn")
                    if s > 0:
                        t2 = scanp.tile([H, 16], f32, tag="t2")
                        nc.vector.tensor_tensor(t2[:], sig[0:20, :], cprev[:], op=ALU.mult)
                        t1 = scanp.tile([H, 16], f32, tag="t1")
                        nc.vector.tensor_tensor(t1[:], sig[64:84, :], gt96[64:84, :], op=ALU.mult)
                        nc.vector.tensor_tensor(cn[:], t1[:], t2[:], op=ALU.add)
                    else:
                        nc.vector.tensor_tensor(cn[:], sig[64:84, :], gt96[64:84, :], op=ALU.mult)
                    tc64 = scanp.tile([64, 16], f32, tag="tc64")
                    nc.scalar.activation(tc64[32:52, :], cn[:], ACTF.Tanh)
                    # h written straight into hs: fwd col r*W+tf, bwd col
                    # W*RPC + r*W + tb; strides: per-col-block delta
                    hdst = apmod(hs[:, 0:16], tf,
                                 [[2 * W * RPC, H], [W * RPC + tb - tf, 2], [W, 8]])
                    nc.vector.tensor_tensor(hdst, sig[32:52, :], tc64[32:52, :],
                                            op=ALU.mult)
                    cprev = cn
                    hprev = {"f": hsfv[:, :, tf], "b": hsbv[:, :, tb]}

                # ================= output: transpose + DMA =================
                for r in range(RPC):
                    for tb_i in range(2):
                        cols = slice(r * W + tb_i * 128, r * W + (tb_i + 1) * 128)
                        colsb = slice(W * RPC + r * W + tb_i * 128,
                                      W * RPC + r * W + (tb_i + 1) * 128)
                        pt = psP.tile([128, 2 * H], f32, tag="pg", bufs=4, name="pt")
                        nc.tensor.matmul(pt[:, 0:H], hs[:, cols], idm[0:H, 0:H],
                                         is_transpose=True, start=True, stop=False,
                                         skip_group_check=True)
                        nc.tensor.matmul(pt[:, H:2 * H], hs[:, colsb], idm[0:H, 0:H],
                                         is_transpose=True, start=False, stop=True,
                                         skip_group_check=True)
                        hsT = scanp.tile([128, 2 * H], f32, tag="hsT")
                        nc.scalar.copy(hsT[:], pt[:])
                        nc.sync.dma_start(
                            out[r, tb_i * 128:(tb_i + 1) * 128, :], hsT[:])

            def body():
                if parts in ("all", "phases"):
                    phases(psP)
                if parts in ("all", "scan"):
                    scan(psP, standalone=(parts == "scan"))

            if reps > 1:
                with tc.For_i(0, reps):
                    for _ in range(nbody):
                        body()
            else:
                body()

    nc.finalize()
    return nc


def _prep_weights(cap_table, w_ih_f, w_hh_f, b_f, w_ih_b, w_hh_b, b_b):
    """Host-side reorder of weights into DMA-friendly layouts (fp32).

    Gate rows are remapped from pytorch order [i,f,g,o] (4x20) to the padded
    device layout [i@0:20, f@32:52, o@64:84, g@96:116] (128 rows).
    """
    dst = np.concatenate([np.arange(64, 84), np.arange(0, 20),
                          np.arange(96, 116), np.arange(32, 52)])  # i,f,g,o lands

    def prep_dir(w_ih, w_hh, b):
        w_ih = np.asarray(w_ih, np.float32)            # [80, 778]
        w_hh = np.asarray(w_hh, np.float32)            # [80, 20]
        b = np.asarray(b, np.float32)                  # [80]
        w_ih_p = np.zeros((PG, IN_DIM), np.float32)
        w_hh_p = np.zeros((PG, H), np.float32)
        b_p = np.zeros(PG, np.float32)
        w_ih_p[dst] = w_ih
        w_hh_p[dst] = w_hh
        b_p[dst] = b
        w_ih_p[96:116] *= 2.0   # tanh(g) via 2*sigmoid(2g) - 1
        w_hh_p[96:116] *= 2.0
        b_p[96:116] *= 2.0
        wihT = np.concatenate([w_ih_p.T, b_p[None, :]], 0)  # [779, 128]
        wihT = np.pad(wihT, ((0, KC * 128 - wihT.shape[0]), (0, 0)))
        chunks = wihT.reshape(KC, 128, PG).transpose(1, 0, 2).reshape(128, KC * PG)
        return chunks, w_hh_p.T.copy()                  # [128, 896], [20, 128]

    cf, hf = prep_dir(w_ih_f, w_hh_f, b_f)
    cb, hb = prep_dir(w_ih_b, w_hh_b, b_b)
    wih_host = np.ascontiguousarray(np.concatenate([cf, cb], axis=1))   # [128, 1792]
    whh_host = np.ascontiguousarray(np.concatenate([hf, hb], axis=1))   # [20, 256]
    ident = np.eye(PG, dtype=np.float32)
    return wih_host, whh_host, ident


def kernel(**inputs) -> np.ndarray:
    from concourse.bass_utils import run_bass_kernel_spmd

    hiddens = np.ascontiguousarray(np.asarray(inputs["hiddens"], np.float32))
    bert2toks = np.asarray(inputs["bert2toks"]).astype(np.int64)
    cap_inds = np.asarray(inputs["cap_inds"]).astype(np.int64)
    cap_table = np.asarray(inputs["cap_table"], np.float32)

    wih_host, whh_host, ident = _prep_weights(
        cap_table, inputs["w_ih_f"], inputs["w_hh_f"], inputs["b_f"],
        inputs["w_ih_b"], inputs["w_hh_b"], inputs["b_b"])

    if "nc" not in _CACHE:
        _CACHE["nc"] = _build_program(hid_external=True, reps=1)
    nc = _CACHE["nc"]

    cap_emb = cap_table[cap_inds]                      # [64, 256, 10]
    in_maps = []
    for c in range(N_CORES):
        rows = slice(c * RPC, (c + 1) * RPC)
        ids = bert2toks[rows]                          # [8, 512]
        idsT = ids.reshape(RPC, 4, 128).transpose(2, 0, 1).reshape(128, RPC * 4)
        capT = np.concatenate(
            [cap_emb[rows].transpose(2, 0, 1).reshape(CAP_DIM, RPC * W),
             np.ones((1, RPC * W), np.float32)], axis=0)
        in_maps.append({
            "hid": np.ascontiguousarray(hiddens[:, rows]),
            "idsT": np.ascontiguousarray(idsT.astype(np.float32)),
            "capT": np.ascontiguousarray(capT.astype(np.float32)),
            "wih": wih_host,
            "whh": whh_host,
            "ident": ident,
        })

    res = run_bass_kernel_spmd(nc, in_maps, list(range(N_CORES)))
    return np.concatenate([res.results[c]["out"] for c in range(N_CORES)],
                          axis=0).astype(np.float32)
